# revision 37
# baseline (speedup 1.0000x reference)
"""ChebConv (order-4) GNN layer on 8 Trainium2 NeuronCores.

Reference computation (fp32):
    T0 = x, T1 = G x, Tk = 2 G T{k-1} - T{k-2}
    out = sum_k Tk @ W[k]          # [N, F] with N=10000, F=32

Strategy (v2 — plain bf16 + 4x col-tiled PE + pinned G):
  * Power basis: y0 = x, yk = G y{k-1}, out = sum_k yk @ Wp[k] with
    Wp = [W0 - W2, W1 - 3 W3, 2 W2, 4 W3] (exact modulo fp reassociation).
  * Everything in plain bf16 (G, x, y between hops, Wp); fp32 PSUM
    accumulation. Measured end-to-end rel err ~4e-3 vs the 2e-2 gate;
    halves DRAM bytes and cuts matmul passes 3x vs the hi/lo split.
  * Row-shard G over 8 cores (1280 cols of G^T per core, padded
    N 10000 -> 10240). Contraction j-chunks: 79 of 80 (last is all-pad).
  * Per hop, y^T accumulates per <=512-col sweep: sweeps (512,512,256).
    Sweeps 0,1 (1024 cols) of G^T stay PINNED in SBUF across all hops
    (~158 KB/partition); sweep 2 streams from DRAM every hop,
    interleaved 2:1 between pinned groups so the DMA stream is consumed
    uniformly across the hop instead of in an end-of-hop burst.
  * Matmuls are 4x column-tiled: groups of 4 j-chunks run concurrently
    in the 128x128 PE array (lhsT = v_j [128,32] at col-group 32t, rhs
    = G^T tile [128,l]), accumulating into 4 disjoint 32-partition
    strips of one PSUM bank. DVE reduces the 4 strips (fp32) and casts
    to bf16 y_t. ~4x PE throughput at M=32.
  * G^T rows are host-permuted into consumption order (parts 0,2,1 to
    match gather completion order), so each 4-j group is one contiguous
    DMA descriptor.
  * The Wp contraction accumulates in PSUM across all hops (k=0 term
    from x^T; one matmul per sweep per hop; stop at k=3), copied to
    SBUF once at the end.
  * After each sweep (hops 1,2), its y rows are PE-transposed to
    natural layout, staged bf16, all-gathered (DRAM bounce), and
    reloaded into the next hop's per-part v tiles via SWDGE (gpsimd)
    so the gather-gated DMA cannot convoy the G stream.
  * Output is returned transposed ([32, 1280] per core); the host
    concatenates, transposes and drops padding.
"""

import sys

if "/opt/trn_rl_repo" not in sys.path:
    sys.path.insert(0, "/opt/trn_rl_repo")

import numpy as np

N = 10000
F = 32
ORDER = 4
NCORES = 8
P = 128
NP = 10240  # padded node count
RPC = NP // NCORES  # cols of G^T per core (1280)
MC = RPC // P  # m-chunks per core (10)
JC_EFF = 79  # 128-row contraction chunks with any real data (80th is pad)
SWEEPS = [(0, 512), (512, 512), (1024, 256)]
PARTS = [(0, 4), (4, 4), (8, 2)]  # (m0, nm) per sweep
CONS_ORDER = [0, 1, 2]  # part consumption order (matches gather arrival)

_CACHE = {}


def _part_of(m):
    for i, (m0, nm) in enumerate(PARTS):
        if m0 <= m < m0 + nm:
            return i
    raise AssertionError


def _jlist_groups():
    """Consumption-ordered j list and 4-j groups (never spanning parts)."""
    jlist = []
    groups = []  # (jstart_idx, size, part)
    for i in CONS_ORDER:
        m0, nm = PARTS[i]
        pj = [
            c * MC + m
            for c in range(NCORES)
            for m in range(m0, m0 + nm)
            if c * MC + m < JC_EFF
        ]
        for a in range(0, len(pj), 4):
            chunk = pj[a : a + 4]
            groups.append((len(jlist) + a, len(chunk), i))
        jlist.extend(pj)
    assert len(jlist) == JC_EFF
    return jlist, groups


JLIST, GROUPS = _jlist_groups()
NG = len(GROUPS)  # 20


def _build():
    import heapq

    from concourse import bacc, mybir, tile

    f32 = mybir.dt.float32
    bf16 = mybir.dt.bfloat16
    vcols = [NCORES * nm * F for (_, nm) in PARTS]

    nc = bacc.Bacc(
        "TRN2", target_bir_lowering=False, debug=False, num_devices=NCORES
    )
    # partition-major: g[p, q*l+col] = G^T[JLIST[q]*128+p, s+col] so every
    # 4-j group is a 4 KB-per-partition contiguous DMA on both sides
    g_dram = [
        nc.dram_tensor(f"g{i}", [P, JC_EFF * l], bf16, kind="ExternalInput").ap()
        for i, (_, l) in enumerate(SWEEPS)
    ]
    xtv = nc.dram_tensor("xtv", [P, sum(vcols)], bf16, kind="ExternalInput").ap()
    xt = nc.dram_tensor("xt", [F, RPC], bf16, kind="ExternalInput").ap()
    wp = nc.dram_tensor("wp", [F, ORDER * F], bf16, kind="ExternalInput").ap()
    ident = nc.dram_tensor("ident", [F, F], bf16, kind="ExternalInput").ap()
    out_t = nc.dram_tensor("outT", [F, RPC], f32, kind="ExternalOutput").ap()

    with tile.TileContext(nc) as tc:
        with (
            tc.tile_pool(name="const", bufs=1) as constp,
            tc.tile_pool(name="g2p", bufs=4) as g2p,
            tc.tile_pool(name="vp", bufs=2) as vp,
            tc.tile_pool(name="sb", bufs=2) as sb,
            tc.tile_pool(name="tmp", bufs=1) as tmpp,
            tc.tile_pool(name="ps_hop", bufs=1, space="PSUM") as ps_hop,
            tc.tile_pool(name="ps_w", bufs=1, space="PSUM") as ps_w,
            tc.tile_pool(name="ps_tp", bufs=2, space="PSUM") as ps_tp,
            tc.tile_pool(name="dram", bufs=2, space="DRAM") as dram,
        ):
            w_sb = constp.tile([F, ORDER * F], bf16)
            nc.scalar.dma_start(w_sb[:], wp)
            id_sb = constp.tile([F, F], bf16)
            nc.scalar.dma_start(id_sb[:], ident)
            xt_sb = constp.tile([F, RPC], bf16)
            nc.scalar.dma_start(xt_sb[:], xt)
            out_sb = constp.tile([F, RPC], f32)
            pins = [
                constp.tile([P, JC_EFF * l], bf16, name=f"pin{i}")
                for i, (_, l) in enumerate(SWEEPS[:2])
            ]

            # initial v (= x) in per-part layout
            v_cur = []
            off = 0
            for i, w_ in enumerate(vcols):
                vt = vp.tile([P, w_], bf16, tag=f"v{i}", name=f"v{i}")
                nc.sync.dma_start(vt[:], xtv[:, off : off + w_])
                off += w_
                v_cur.append(vt)

            def v_of(vtiles, j):
                c, m = j // MC, j % MC
                i = _part_of(m)
                m0, nm = PARTS[i]
                col = (c * nm + (m - m0)) * F
                return vtiles[i][:, col : col + F]

            # Wp output accumulators: one PSUM bank per sweep range, one
            # accumulation group spanning the whole kernel (k = 0..3)
            pw = [
                ps_w.tile([F, l], f32, tag=f"pw{i}", name=f"pw{i}")
                for i, (_, l) in enumerate(SWEEPS)
            ]
            for i, (s, l) in enumerate(SWEEPS):
                nc.tensor.matmul(
                    pw[i][:], lhsT=w_sb[:, 0:F], rhs=xt_sb[:, s : s + l],
                    start=True, stop=False,
                )

            # ---- slot machine: pending epilogues fire by slot index ----
            slot = 0
            seq = 0
            pending = []  # heap of (due_slot, seq, fn)

            def queue(due, fn):
                nonlocal seq
                heapq.heappush(pending, (due, seq, fn))
                seq += 1

            def flush(limit=None):
                while pending and (limit is None or pending[0][0] <= limit):
                    _, _, fn = heapq.heappop(pending)
                    fn()

            def emit_group(hp_, vtiles, i, g, gt=None):
                s, l = SWEEPS[i]
                jstart, sz, _ = GROUPS[g]
                for t in range(sz):
                    q = jstart + t
                    if gt is None:
                        rhs = pins[i][:, q * l : (q + 1) * l]
                    else:
                        rhs = gt[:, t * l : (t + 1) * l]
                    nc.tensor.matmul(
                        hp_[i][32 * t : 32 * (t + 1), 0:l],
                        lhsT=v_of(vtiles, JLIST[q]),
                        rhs=rhs,
                        start=(g == 0),
                        stop=(g == NG - 1),
                        tile_position=(0, 32 * t),
                        skip_group_check=True,
                    )

            def epi_a(i, hp_, y_t_):
                s, l = SWEEPS[i]
                yt_f = tmpp.tile([F, l], f32, tag=f"ytmp{i}", name=f"ytmp{i}")
                nc.vector.tensor_copy(yt_f[:], hp_[i][0:32, 0:l])
                for t in range(1, 4):
                    nc.vector.tensor_add(
                        yt_f[:], yt_f[:], hp_[i][32 * t : 32 * (t + 1), 0:l]
                    )
                nc.vector.tensor_copy(y_t_[:, s : s + l], yt_f[:])

            def epi_b(i, kk, y_t_, stages_):
                s, l = SWEEPS[i]
                if stages_ is not None:
                    m0, nm = PARTS[i]
                    for ml in range(nm):
                        m = m0 + ml
                        tp = ps_tp.tile([P, F], bf16, tag="tp", name="tp")
                        nc.tensor.transpose(
                            tp[:], y_t_[:, m * P : (m + 1) * P], id_sb[:]
                        )
                        nc.vector.tensor_copy(
                            stages_[i][:, ml * F : (ml + 1) * F], tp[:]
                        )
                nc.tensor.matmul(
                    pw[i][:],
                    lhsT=w_sb[:, kk * F : (kk + 1) * F],
                    rhs=y_t_[:, s : s + l],
                    start=False,
                    stop=(kk == ORDER - 1),
                )
                if kk == ORDER - 1:
                    # final hop: write this column range back immediately
                    nc.vector.tensor_copy(out_sb[:, s : s + l], pw[i][:])
                    nc.scalar.dma_start(out_t[:, s : s + l], out_sb[:, s : s + l])

            def epi_c(i, v_next_, stages_, reloads_):
                nm = PARTS[i][1]
                cc_in = dram.tile(
                    [P, nm * F], bf16, tag=f"ccin{i}", name=f"ccin{i}"
                )
                # SWDGE: the sync/scalar queues carry a deep G backlog in
                # hop 1, which would delay this small write by ~30us
                nc.gpsimd.dma_start(cc_in[:], stages_[i][:])
                # Shared-output AllGather rides the fast path (~5us at this
                # size vs 10-30us for the Local-output mesh bounce)
                cc_out = dram.tile(
                    [NCORES * P, nm * F], bf16, tag=f"ccout{i}",
                    name=f"ccout{i}", addr_space="Shared",
                )
                nc.gpsimd.collective_compute(
                    "AllGather",
                    mybir.AluOpType.bypass,
                    replica_groups=[list(range(NCORES))],
                    ins=[cc_in.opt()],
                    outs=[cc_out.opt()],
                )

                def reload(i=i, cc_out=cc_out):
                    nc.gpsimd.dma_start(
                        v_next_[i][:].rearrange("p (c m) -> p c m", c=NCORES),
                        cc_out[:].rearrange("(c p) m -> p c m", p=P),
                    )

                reloads_.append(reload)

            def sweep_done(i, kk, hp_, y_t_, v_next_, stages_, reloads_):
                # reduce now; transposes/Wp/gather-trigger shortly after;
                # reloads are deferred to hop end so they cannot delay later
                # gather triggers on the gpsimd queue
                epi_a(i, hp_, y_t_)
                st = stages_ if kk < ORDER - 1 else None
                d = 1 if kk == 1 else 2
                queue(
                    slot + d,
                    lambda i=i, kk=kk, y=y_t_, st=st: epi_b(i, kk, y, st),
                )
                if kk < ORDER - 1:
                    queue(
                        slot + d,
                        lambda i=i, vn=v_next_, st=st, r=reloads_: epi_c(
                            i, vn, st, r
                        ),
                    )

            for k in range(1, ORDER):
                y_t = sb.tile([F, RPC], bf16, tag="yt", name=f"yt{k}")
                hp = [
                    ps_hop.tile([P, l], f32, tag=f"hp{i}", name=f"hp{i}k{k}")
                    for i, (_, l) in enumerate(SWEEPS)
                ]
                last = k == ORDER - 1
                reloads = []
                vn = stg = None
                if not last:
                    vn = [
                        vp.tile([P, w_], bf16, tag=f"v{i}", name=f"vn{i}k{k}")
                        for i, w_ in enumerate(vcols)
                    ]
                    stg = [
                        sb.tile(
                            [P, nm * F], bf16, tag=f"stage{i}", name=f"st{i}k{k}"
                        )
                        for i, (_, nm) in enumerate(PARTS)
                    ]

                # slot schedule: hop 1 runs all pinned groups first (the pin
                # stream alone then gates sweep-0/1 completion, so their
                # gathers fire ~20us earlier) and the streamed sweep as a
                # tail; hops 2/3 interleave 4 pinned + 1 streamed per tri so
                # the stream is consumed uniformly
                if k == 1:
                    sched = [("P", g) for g in range(2 * NG)]
                    sched += [("S", g) for g in range(NG)]
                else:
                    sched = []
                    for tri in range(NG):
                        for _ in range(4 if tri < NG // 2 else 0):
                            sched.append(("P", len([x for x in sched if x[0] == "P"])))
                        sched.append(("S", tri))
                gt_pair = None
                for si, (kind, g) in enumerate(sched):
                    if kind == "P":
                        pg = g
                        i, g = (0, pg) if pg < NG else (1, pg - NG)
                        s, l = SWEEPS[i]
                        jstart, sz, _ = GROUPS[g]
                        if k == 1 and pg % 2 == 0:
                            # 1 MiB descriptors (two 4-j groups), balanced
                            # across both HWDGE queues: >=1MiB per dma_start
                            # is needed for peak DMA efficiency
                            j2, s2_, _ = GROUPS[g + 1]
                            hi = (j2 + s2_) * l
                            eng = nc.scalar if (pg // 2) % 2 else nc.sync
                            eng.dma_start(
                                pins[i][:, jstart * l : hi],
                                g_dram[i][:, jstart * l : hi],
                            )
                        emit_group(hp, v_cur, i, g)
                        slot += 1
                        if g == NG - 1:
                            sweep_done(i, k, hp, y_t, vn, stg, reloads)
                        flush(slot)
                    else:
                        s2, l2 = SWEEPS[2]
                        jstart, sz, _ = GROUPS[g]
                        if g % 2 == 0:
                            # 512 KB descriptors covering two stream groups
                            j2, s2_, _ = GROUPS[g + 1]
                            w2 = (j2 + s2_) * l2 - jstart * l2
                            gt_pair = g2p.tile(
                                [P, 8 * l2], bf16, tag="g2", name=f"g2k{k}"
                            )
                            s_eng = nc.sync if (g // 2) % 2 else nc.scalar
                            s_eng.dma_start(
                                gt_pair[:, 0:w2],
                                g_dram[2][:, jstart * l2 : jstart * l2 + w2],
                            )
                            gt_off = jstart * l2
                        gt = gt_pair[:, jstart * l2 - gt_off :]
                        emit_group(hp, v_cur, 2, g, gt=gt)
                        slot += 1
                        if g == NG - 1:
                            sweep_done(2, k, hp, y_t, vn, stg, reloads)
                        flush(slot)
                flush()  # hop-k epilogues before the next hop's matmuls
                for r in reloads:
                    r()
                if not last:
                    v_cur = vn

            flush()  # remaining epilogues (hop-3 tail)

    nc.compile()
    return nc


def get_nc():
    if "nc" not in _CACHE:
        _CACHE["nc"] = _build()
    return _CACHE["nc"]


def prep_inputs(x, gso, weight):
    """Host-side shard prep. Returns in_maps for run_bass_kernel_spmd."""
    import ml_dtypes

    bf = ml_dtypes.bfloat16
    n = x.shape[0]
    x = np.asarray(x, dtype=np.float32)
    gso = np.asarray(gso, dtype=np.float32)
    weight = np.asarray(weight, dtype=np.float32)

    wp = np.concatenate(
        [
            weight[0] - weight[2],
            weight[1] - 3.0 * weight[3],
            2.0 * weight[2],
            4.0 * weight[3],
        ],
        axis=1,
    ).astype(bf)  # [F, ORDER*F]

    xpad = np.zeros((NP, F), dtype=np.float32)
    xpad[:n] = x
    x_bf = xpad.astype(bf)
    gpad = np.zeros((NP, NP), dtype=np.float32)
    gpad[:n, :n] = gso
    g_bf = gpad.astype(bf)

    jrows = np.concatenate(
        [np.arange(j * P, (j + 1) * P) for j in JLIST]
    )  # [JC_EFF*P]

    # x in per-part v layout: part i -> [P, (c, ml, f)]
    xr = x_bf.reshape(NCORES, MC, P, F)
    xtv = np.concatenate(
        [
            np.ascontiguousarray(
                xr[:, m0 : m0 + nm].transpose(2, 0, 1, 3)
            ).reshape(P, NCORES * nm * F)
            for (m0, nm) in PARTS
        ],
        axis=1,
    )

    ident = np.eye(F, dtype=bf)
    in_maps = []
    for c in range(NCORES):
        rows = slice(c * RPC, (c + 1) * RPC)
        # G^T block, contraction rows permuted into consumption order and
        # stored partition-major: g[p, q*l+col] = G^T[jrows[q*128+p], col]
        ght_p = g_bf[rows][:, jrows].T.reshape(JC_EFF, P, RPC)
        ght_p = np.ascontiguousarray(ght_p.transpose(1, 0, 2))  # [P, JC_EFF, RPC]
        m = {"xtv": xtv, "wp": wp, "ident": ident}
        m["xt"] = np.ascontiguousarray(x_bf[rows].T)  # [F, RPC]
        for i, (s, l) in enumerate(SWEEPS):
            m[f"g{i}"] = np.ascontiguousarray(ght_p[:, :, s : s + l]).reshape(
                P, JC_EFF * l
            )
        in_maps.append(m)
    return in_maps


def assemble_output(results, n=N, ncores=NCORES):
    out_t = np.concatenate([results[c]["outT"] for c in range(ncores)], axis=1)
    return np.ascontiguousarray(out_t.T[:n]).astype(np.float32)


def kernel(x, gso, weight):
    import time

    from concourse import bass_utils

    nc = get_nc()
    in_maps = prep_inputs(x, gso, weight)
    last_err = None
    for attempt in range(3):
        try:
            res = bass_utils.run_bass_kernel_spmd(
                nc, in_maps, core_ids=list(range(NCORES))
            )
            return assemble_output(res.results)
        except Exception as e:  # transient device wedge: retry
            last_err = e
            time.sleep(5.0 * (attempt + 1))
    raise last_err


# revision 39
# speedup vs baseline: 1.0182x; 1.0182x over previous
"""ChebConv (order-4) GNN layer on 8 Trainium2 NeuronCores.

Reference computation (fp32):
    T0 = x, T1 = G x, Tk = 2 G T{k-1} - T{k-2}
    out = sum_k Tk @ W[k]          # [N, F] with N=10000, F=32

Strategy (plain bf16 + 4x col-tiled PE + pinned G + fast gathers):
  * Power basis: y0 = x, yk = G y{k-1}, out = sum_k yk @ Wp[k] with
    Wp = [W0 - W2, W1 - 3 W3, 2 W2, 4 W3] (exact modulo fp reassociation).
  * Everything in plain bf16 (G, x, y between hops, Wp); fp32 PSUM
    accumulation. Measured end-to-end rel err ~4.4e-3 vs the 2e-2 gate;
    halves DRAM bytes and cuts matmul passes 3x vs an fp32-accurate
    hi/lo split.
  * Row-shard G over 8 cores (1280 cols of G^T per core, padded
    N 10000 -> 10240). Contraction j-chunks: 79 of 80 (last is all-pad).
  * Per hop, y^T accumulates per <=512-col sweep: sweeps (512,512,256).
    Sweeps 0,1 (1024 cols) of G^T stay PINNED in SBUF across all hops
    (~158 KB/partition); sweep 2 streams from DRAM every hop. In hop 1
    the pinned groups run first (the pin stream alone gates sweep-0/1
    completion so their gathers fire ~20us earlier) with the streamed
    sweep as a tail; hops 2/3 interleave 4 pinned + 1 streamed groups
    so the stream is consumed uniformly across the hop.
  * Matmuls are 4x column-tiled: groups of 4 j-chunks run concurrently
    in the 128x128 PE array (lhsT = v_j [128,32] at col-group 32t, rhs
    = G^T tile [128,l]), accumulating into 4 disjoint 32-partition
    strips of one PSUM bank (skip_group_check: the sim's zero-region
    tracker ignores partition offsets; per-partition has_written bits
    make disjoint strips safe). DVE reduces the 4 strips in fp32 and
    casts to bf16 y_t. ~4x PE throughput at M=32.
  * G^T is stored partition-major and row-permuted into consumption
    order on the host, so every pair of 4-j groups is one contiguous
    >=1MiB / 512KB DMA descriptor (needed for peak DMA efficiency),
    alternated across both HWDGE queues (sync/scalar) to balance bytes.
  * The Wp contraction accumulates in PSUM across all hops (k=0 term
    from x^T; one matmul per sweep per hop; stop at k=3); each column
    range is written back right after its final-hop matmul.
  * After each sweep (hops 1,2), its y rows are PE-transposed to
    natural layout, staged bf16, and all-gathered with a Shared-output
    AllGather (the Local-output mesh path is 2-4x slower). Gather
    triggers are async on the gpsimd queue; the reloads into the next
    hop's per-part v tiles (SWDGE) are deferred to hop end so a reload
    waiting on gather data cannot delay later gather triggers.
  * Consumption (j-group) order = part production order [0,1,2], so
    each hop starts on j-chunks whose gather completed first.
  * Output is returned transposed ([32, 1280] per core); the host
    concatenates, transposes and drops padding.
"""

import sys

if "/opt/trn_rl_repo" not in sys.path:
    sys.path.insert(0, "/opt/trn_rl_repo")

import numpy as np

N = 10000
F = 32
ORDER = 4
NCORES = 8
P = 128
NP = 10240  # padded node count
RPC = NP // NCORES  # cols of G^T per core (1280)
MC = RPC // P  # m-chunks per core (10)
JC_EFF = 79  # 128-row contraction chunks with any real data (80th is pad)
SWEEPS = [(0, 512), (512, 512), (1024, 256)]
PARTS = [(0, 4), (4, 4), (8, 2)]  # (m0, nm) per sweep
CONS_ORDER = [0, 1, 2]  # part consumption order (matches gather arrival)

_CACHE = {}


def _part_of(m):
    for i, (m0, nm) in enumerate(PARTS):
        if m0 <= m < m0 + nm:
            return i
    raise AssertionError


def _jlist_groups():
    """Consumption-ordered j list and 4-j groups (never spanning parts)."""
    jlist = []
    groups = []  # (jstart_idx, size, part)
    for i in CONS_ORDER:
        m0, nm = PARTS[i]
        pj = [
            c * MC + m
            for c in range(NCORES)
            for m in range(m0, m0 + nm)
            if c * MC + m < JC_EFF
        ]
        for a in range(0, len(pj), 4):
            chunk = pj[a : a + 4]
            groups.append((len(jlist) + a, len(chunk), i))
        jlist.extend(pj)
    assert len(jlist) == JC_EFF
    return jlist, groups


JLIST, GROUPS = _jlist_groups()
NG = len(GROUPS)  # 20


def _build():
    import heapq

    from concourse import bacc, mybir, tile

    f32 = mybir.dt.float32
    bf16 = mybir.dt.bfloat16
    vcols = [NCORES * nm * F for (_, nm) in PARTS]

    nc = bacc.Bacc(
        "TRN2", target_bir_lowering=False, debug=False, num_devices=NCORES
    )
    # partition-major: g[p, q*l+col] = G^T[JLIST[q]*128+p, s+col] so every
    # 4-j group is a 4 KB-per-partition contiguous DMA on both sides
    g_dram = [
        nc.dram_tensor(f"g{i}", [P, JC_EFF * l], bf16, kind="ExternalInput").ap()
        for i, (_, l) in enumerate(SWEEPS)
    ]
    xtv = nc.dram_tensor("xtv", [P, sum(vcols)], bf16, kind="ExternalInput").ap()
    xt = nc.dram_tensor("xt", [F, RPC], bf16, kind="ExternalInput").ap()
    wp = nc.dram_tensor("wp", [F, ORDER * F], bf16, kind="ExternalInput").ap()
    ident = nc.dram_tensor("ident", [F, F], bf16, kind="ExternalInput").ap()
    out_t = nc.dram_tensor("outT", [F, RPC], f32, kind="ExternalOutput").ap()

    with tile.TileContext(nc) as tc:
        with (
            tc.tile_pool(name="const", bufs=1) as constp,
            tc.tile_pool(name="g2p", bufs=4) as g2p,
            tc.tile_pool(name="vp", bufs=2) as vp,
            tc.tile_pool(name="sb", bufs=2) as sb,
            tc.tile_pool(name="tmp", bufs=1) as tmpp,
            tc.tile_pool(name="ps_hop", bufs=1, space="PSUM") as ps_hop,
            tc.tile_pool(name="ps_w", bufs=1, space="PSUM") as ps_w,
            tc.tile_pool(name="ps_tp", bufs=2, space="PSUM") as ps_tp,
            tc.tile_pool(name="dram", bufs=2, space="DRAM") as dram,
        ):
            w_sb = constp.tile([F, ORDER * F], bf16)
            nc.scalar.dma_start(w_sb[:], wp)
            id_sb = constp.tile([F, F], bf16)
            nc.scalar.dma_start(id_sb[:], ident)
            xt_sb = constp.tile([F, RPC], bf16)
            nc.scalar.dma_start(xt_sb[:], xt)
            out_sb = constp.tile([F, RPC], f32)
            pins = [
                constp.tile([P, JC_EFF * l], bf16, name=f"pin{i}")
                for i, (_, l) in enumerate(SWEEPS[:2])
            ]

            # initial v (= x) in per-part layout
            v_cur = []
            off = 0
            for i, w_ in enumerate(vcols):
                vt = vp.tile([P, w_], bf16, tag=f"v{i}", name=f"v{i}")
                nc.sync.dma_start(vt[:], xtv[:, off : off + w_])
                off += w_
                v_cur.append(vt)

            def v_of(vtiles, j):
                c, m = j // MC, j % MC
                i = _part_of(m)
                m0, nm = PARTS[i]
                col = (c * nm + (m - m0)) * F
                return vtiles[i][:, col : col + F]

            # Wp output accumulators: one PSUM bank per sweep range, one
            # accumulation group spanning the whole kernel (k = 0..3)
            pw = [
                ps_w.tile([F, l], f32, tag=f"pw{i}", name=f"pw{i}")
                for i, (_, l) in enumerate(SWEEPS)
            ]
            for i, (s, l) in enumerate(SWEEPS):
                nc.tensor.matmul(
                    pw[i][:], lhsT=w_sb[:, 0:F], rhs=xt_sb[:, s : s + l],
                    start=True, stop=False,
                )

            # ---- slot machine: pending epilogues fire by slot index ----
            slot = 0
            seq = 0
            pending = []  # heap of (due_slot, seq, fn)

            def queue(due, fn):
                nonlocal seq
                heapq.heappush(pending, (due, seq, fn))
                seq += 1

            def flush(limit=None):
                while pending and (limit is None or pending[0][0] <= limit):
                    _, _, fn = heapq.heappop(pending)
                    fn()

            def emit_group(hp_, vtiles, i, g, gt=None):
                s, l = SWEEPS[i]
                jstart, sz, _ = GROUPS[g]
                for t in range(sz):
                    q = jstart + t
                    if gt is None:
                        rhs = pins[i][:, q * l : (q + 1) * l]
                    else:
                        rhs = gt[:, t * l : (t + 1) * l]
                    nc.tensor.matmul(
                        hp_[i][32 * t : 32 * (t + 1), 0:l],
                        lhsT=v_of(vtiles, JLIST[q]),
                        rhs=rhs,
                        start=(g == 0),
                        stop=(g == NG - 1),
                        tile_position=(0, 32 * t),
                        skip_group_check=True,
                    )

            def epi_a(i, hp_, y_t_):
                s, l = SWEEPS[i]
                yt_f = tmpp.tile([F, l], f32, tag=f"ytmp{i}", name=f"ytmp{i}")
                nc.vector.tensor_copy(yt_f[:], hp_[i][0:32, 0:l])
                for t in range(1, 4):
                    nc.vector.tensor_add(
                        yt_f[:], yt_f[:], hp_[i][32 * t : 32 * (t + 1), 0:l]
                    )
                nc.vector.tensor_copy(y_t_[:, s : s + l], yt_f[:])

            def epi_b(i, kk, y_t_, stages_):
                s, l = SWEEPS[i]
                if stages_ is not None:
                    m0, nm = PARTS[i]
                    for ml in range(nm):
                        m = m0 + ml
                        tp = ps_tp.tile([P, F], bf16, tag="tp", name="tp")
                        nc.tensor.transpose(
                            tp[:], y_t_[:, m * P : (m + 1) * P], id_sb[:]
                        )
                        nc.vector.tensor_copy(
                            stages_[i][:, ml * F : (ml + 1) * F], tp[:]
                        )
                nc.tensor.matmul(
                    pw[i][:],
                    lhsT=w_sb[:, kk * F : (kk + 1) * F],
                    rhs=y_t_[:, s : s + l],
                    start=False,
                    stop=(kk == ORDER - 1),
                )
                if kk == ORDER - 1:
                    # final hop: write this column range back immediately
                    nc.vector.tensor_copy(out_sb[:, s : s + l], pw[i][:])
                    nc.scalar.dma_start(out_t[:, s : s + l], out_sb[:, s : s + l])

            def epi_c(i, v_next_, stages_, reloads_):
                nm = PARTS[i][1]
                cc_in = dram.tile(
                    [P, nm * F], bf16, tag=f"ccin{i}", name=f"ccin{i}"
                )
                nc.sync.dma_start(cc_in[:], stages_[i][:])
                # Shared-output AllGather rides the fast path (~5us at this
                # size vs 10-30us for the Local-output mesh bounce)
                cc_out = dram.tile(
                    [NCORES * P, nm * F], bf16, tag=f"ccout{i}",
                    name=f"ccout{i}", addr_space="Shared",
                )
                nc.gpsimd.collective_compute(
                    "AllGather",
                    mybir.AluOpType.bypass,
                    replica_groups=[list(range(NCORES))],
                    ins=[cc_in.opt()],
                    outs=[cc_out.opt()],
                )

                def reload(i=i, cc_out=cc_out):
                    nc.gpsimd.dma_start(
                        v_next_[i][:].rearrange("p (c m) -> p c m", c=NCORES),
                        cc_out[:].rearrange("(c p) m -> p c m", p=P),
                    )

                reloads_.append(reload)

            def sweep_done(i, kk, hp_, y_t_, v_next_, stages_, reloads_):
                # reduce now; transposes/Wp/gather-trigger shortly after;
                # reloads are deferred to hop end so they cannot delay later
                # gather triggers on the gpsimd queue
                epi_a(i, hp_, y_t_)
                st = stages_ if kk < ORDER - 1 else None
                d = 1 if kk == 1 else 2
                queue(
                    slot + d,
                    lambda i=i, kk=kk, y=y_t_, st=st: epi_b(i, kk, y, st),
                )
                if kk < ORDER - 1:
                    queue(
                        slot + d,
                        lambda i=i, vn=v_next_, st=st, r=reloads_: epi_c(
                            i, vn, st, r
                        ),
                    )

            for k in range(1, ORDER):
                y_t = sb.tile([F, RPC], bf16, tag="yt", name=f"yt{k}")
                hp = [
                    ps_hop.tile([P, l], f32, tag=f"hp{i}", name=f"hp{i}k{k}")
                    for i, (_, l) in enumerate(SWEEPS)
                ]
                last = k == ORDER - 1
                reloads = []
                vn = stg = None
                if not last:
                    vn = [
                        vp.tile([P, w_], bf16, tag=f"v{i}", name=f"vn{i}k{k}")
                        for i, w_ in enumerate(vcols)
                    ]
                    stg = [
                        sb.tile(
                            [P, nm * F], bf16, tag=f"stage{i}", name=f"st{i}k{k}"
                        )
                        for i, (_, nm) in enumerate(PARTS)
                    ]

                # slot schedule: hop 1 runs all pinned groups first (the pin
                # stream alone then gates sweep-0/1 completion, so their
                # gathers fire ~20us earlier) and the streamed sweep as a
                # tail; hops 2/3 interleave 4 pinned + 1 streamed per tri so
                # the stream is consumed uniformly
                if k == 1:
                    sched = [("P", g) for g in range(2 * NG)]
                    sched += [("S", g) for g in range(NG)]
                else:
                    sched = []
                    for tri in range(NG):
                        for _ in range(4 if tri < NG // 2 else 0):
                            sched.append(("P", len([x for x in sched if x[0] == "P"])))
                        sched.append(("S", tri))
                gt_pair = None
                for si, (kind, g) in enumerate(sched):
                    if kind == "P":
                        pg = g
                        i, g = (0, pg) if pg < NG else (1, pg - NG)
                        s, l = SWEEPS[i]
                        jstart, sz, _ = GROUPS[g]
                        if k == 1 and pg % 2 == 0:
                            # 1 MiB descriptors (two 4-j groups), balanced
                            # across both HWDGE queues: >=1MiB per dma_start
                            # is needed for peak DMA efficiency
                            j2, s2_, _ = GROUPS[g + 1]
                            hi = (j2 + s2_) * l
                            eng = nc.scalar if (pg // 2) % 2 else nc.sync
                            eng.dma_start(
                                pins[i][:, jstart * l : hi],
                                g_dram[i][:, jstart * l : hi],
                            )
                        emit_group(hp, v_cur, i, g)
                        slot += 1
                        if g == NG - 1:
                            sweep_done(i, k, hp, y_t, vn, stg, reloads)
                        flush(slot)
                    else:
                        s2, l2 = SWEEPS[2]
                        jstart, sz, _ = GROUPS[g]
                        if g % 2 == 0:
                            # 512 KB descriptors covering two stream groups
                            j2, s2_, _ = GROUPS[g + 1]
                            w2 = (j2 + s2_) * l2 - jstart * l2
                            gt_pair = g2p.tile(
                                [P, 8 * l2], bf16, tag="g2", name=f"g2k{k}"
                            )
                            s_eng = nc.sync if (g // 2) % 2 else nc.scalar
                            s_eng.dma_start(
                                gt_pair[:, 0:w2],
                                g_dram[2][:, jstart * l2 : jstart * l2 + w2],
                            )
                            gt_off = jstart * l2
                        gt = gt_pair[:, jstart * l2 - gt_off :]
                        emit_group(hp, v_cur, 2, g, gt=gt)
                        slot += 1
                        if g == NG - 1:
                            sweep_done(2, k, hp, y_t, vn, stg, reloads)
                        flush(slot)
                flush()  # hop-k epilogues before the next hop's matmuls
                for r in reloads:
                    r()
                if not last:
                    v_cur = vn

            flush()  # remaining epilogues (hop-3 tail)

    nc.compile()
    return nc


def get_nc():
    if "nc" not in _CACHE:
        _CACHE["nc"] = _build()
    return _CACHE["nc"]


def prep_inputs(x, gso, weight):
    """Host-side shard prep. Returns in_maps for run_bass_kernel_spmd."""
    import ml_dtypes

    bf = ml_dtypes.bfloat16
    n = x.shape[0]
    x = np.asarray(x, dtype=np.float32)
    gso = np.asarray(gso, dtype=np.float32)
    weight = np.asarray(weight, dtype=np.float32)

    wp = np.concatenate(
        [
            weight[0] - weight[2],
            weight[1] - 3.0 * weight[3],
            2.0 * weight[2],
            4.0 * weight[3],
        ],
        axis=1,
    ).astype(bf)  # [F, ORDER*F]

    xpad = np.zeros((NP, F), dtype=np.float32)
    xpad[:n] = x
    x_bf = xpad.astype(bf)
    gpad = np.zeros((NP, NP), dtype=np.float32)
    gpad[:n, :n] = gso
    g_bf = gpad.astype(bf)

    jrows = np.concatenate(
        [np.arange(j * P, (j + 1) * P) for j in JLIST]
    )  # [JC_EFF*P]

    # x in per-part v layout: part i -> [P, (c, ml, f)]
    xr = x_bf.reshape(NCORES, MC, P, F)
    xtv = np.concatenate(
        [
            np.ascontiguousarray(
                xr[:, m0 : m0 + nm].transpose(2, 0, 1, 3)
            ).reshape(P, NCORES * nm * F)
            for (m0, nm) in PARTS
        ],
        axis=1,
    )

    ident = np.eye(F, dtype=bf)
    in_maps = []
    for c in range(NCORES):
        rows = slice(c * RPC, (c + 1) * RPC)
        # G^T block, contraction rows permuted into consumption order and
        # stored partition-major: g[p, q*l+col] = G^T[jrows[q*128+p], col]
        ght_p = g_bf[rows][:, jrows].T.reshape(JC_EFF, P, RPC)
        ght_p = np.ascontiguousarray(ght_p.transpose(1, 0, 2))  # [P, JC_EFF, RPC]
        m = {"xtv": xtv, "wp": wp, "ident": ident}
        m["xt"] = np.ascontiguousarray(x_bf[rows].T)  # [F, RPC]
        for i, (s, l) in enumerate(SWEEPS):
            m[f"g{i}"] = np.ascontiguousarray(ght_p[:, :, s : s + l]).reshape(
                P, JC_EFF * l
            )
        in_maps.append(m)
    return in_maps


def assemble_output(results, n=N, ncores=NCORES):
    out_t = np.concatenate([results[c]["outT"] for c in range(ncores)], axis=1)
    return np.ascontiguousarray(out_t.T[:n]).astype(np.float32)


def kernel(x, gso, weight):
    import time

    from concourse import bass_utils

    nc = get_nc()
    in_maps = prep_inputs(x, gso, weight)
    last_err = None
    for attempt in range(3):
        try:
            res = bass_utils.run_bass_kernel_spmd(
                nc, in_maps, core_ids=list(range(NCORES))
            )
            return assemble_output(res.results)
        except Exception as e:  # transient device wedge: retry
            last_err = e
            time.sleep(5.0 * (attempt + 1))
    raise last_err
